# revision 8
# baseline (speedup 1.0000x reference)
"""Grouped SwiGLU MoE MLP (16 experts) on 8 NeuronCores, expert-parallel.

Reference computation, per expert e over its contiguous token slice xi:
    out = (silu(xi @ w_gate[e].T) * (xi @ w_up[e].T)) @ w_down[e].T

Sharding: expert-parallel. Core c owns experts {2c, 2c+1}; the host hands it
the matching contiguous 2048-token slice of x (tokens are pre-sorted by
expert), so no device-side collectives are needed. Everything is handed to
the device feature-major (transposed on host) so the token axis is the
matmul moving/free dimension:

  gateT[f,t] = sum_h wgT[h,f] * xT[h,t]      (PE: lhsT=wgT tile, rhs=xT)
  hidT[f,t]  = silu(gateT) * upT             (ACT silu + DVE mul)
  outT[h,t]  = sum_f wdT[f,h] * hidT[f,t]    (PE: lhsT=wdT tile, rhs=hidT)

float32r matmuls (full PE rate at N>=512 free dim), fp32 PSUM accumulation.
Weights stream through SBUF in >=1 MiB chunks; x and hidden stay resident.
"""

import numpy as np

import concourse.bass as bass
import concourse.bacc as bacc
import concourse.mybir as mybir
from concourse import tile
from concourse.bass_utils import run_bass_kernel_spmd

E, T, H, F = 16, 16384, 1024, 2048
NCORES = 8
EPC = E // NCORES          # experts per core
TPE = T // E               # tokens per expert (uniform fast path)
P = 128                    # SBUF partitions
HT = H // P                # 8 h-tiles (contraction tiles for gate/up)
FT = F // P                # 16 f-tiles
HGS = H // P               # 8 output h-groups for down proj
NT = 512                   # matmul moving free dim (PSUM bank = 512 fp32)
TH = TPE // NT             # 2 t-halves
FG = 8                     # f-groups for gate/up weight streaming
FPG = FT // FG             # f-tiles per group = 2
FGW = F // FG              # f columns per group = 256

_F32 = mybir.dt.float32
_F32R = mybir.dt.float32r

_CACHE = {}

# Set by run for test harness introspection (exec_time_ns, profile).
LAST_RESULTS = None
TRACE = False
TRACE_KW = {}
# "silu" uses the native ScalarE Silu LUT; "sigmoid" decomposes it as
# gate*sigmoid(gate) for CoreSim, which lacks a Silu implementation.
ACT_MODE = "silu"


def _build_nc():
    nc = bacc.Bacc()
    xt_d = nc.dram_tensor("xt", [EPC, H, TPE], _F32R, kind="ExternalInput")
    wg_d = nc.dram_tensor("wg", [EPC, H, F], _F32R, kind="ExternalInput")
    wu_d = nc.dram_tensor("wu", [EPC, H, F], _F32R, kind="ExternalInput")
    wd_d = nc.dram_tensor("wd", [EPC, F, H], _F32R, kind="ExternalInput")
    out_d = nc.dram_tensor("outT", [EPC, H, TPE], _F32, kind="ExternalOutput")

    with tile.TileContext(nc) as tc:
        with (
            tc.tile_pool(name="xp", bufs=8) as xp,
            tc.tile_pool(name="wgp", bufs=3) as wgp,
            tc.tile_pool(name="wup", bufs=3) as wup,
            tc.tile_pool(name="wdp", bufs=3) as wdp,
            tc.tile_pool(name="hid", bufs=FT + 1) as hidp,
            tc.tile_pool(name="tmp", bufs=3) as tmpp,
            tc.tile_pool(name="osb", bufs=3) as osbp,
            tc.tile_pool(name="ps", bufs=8, space=bass.MemorySpace.PSUM) as psp,
        ):
            for el in range(EPC):
                # Token activations, resident for the whole expert:
                # 8 tiles [128h, 1024t].
                xts = []
                for ht in range(HT):
                    xt = xp.tile([P, TPE], _F32R, tag="xt")
                    nc.sync.dma_start(xt[:], xt_d[el, ht * P:(ht + 1) * P, :])
                    xts.append(xt)

                hidden = [hidp.tile([P, TPE], _F32R, tag="hid", name=f"hid{el}_{i}") for i in range(FT)]

                # DRAM views with the h-tile index split out of the partition
                # axis: [128p, HT, F].
                wg_v = wg_d[el].rearrange("(a p) f -> p a f", p=P)
                wu_v = wu_d[el].rearrange("(a p) f -> p a f", p=P)

                for fgi in range(FG):
                    fsl = slice(fgi * FGW, (fgi + 1) * FGW)
                    wgt = wgp.tile([P, HT, FGW], _F32R, tag="wg")
                    nc.sync.dma_start(wgt[:], wg_v[:, :, fsl])
                    wut = wup.tile([P, HT, FGW], _F32R, tag="wu")
                    nc.sync.dma_start(wut[:], wu_v[:, :, fsl])

                    gate_ps, up_ps = {}, {}
                    for wt, store in ((wgt, gate_ps), (wut, up_ps)):
                        for ftl in range(FPG):
                            for th in range(TH):
                                store[ftl, th] = psp.tile([P, NT], _F32, tag="ps", name="gu_ps")
                            for ht in range(HT):
                                lhsT = wt[:, ht, ftl * P:(ftl + 1) * P]
                                for th in range(TH):
                                    nc.tensor.matmul(
                                        store[ftl, th][:],
                                        lhsT,
                                        xts[ht][:, th * NT:(th + 1) * NT],
                                        start=(ht == 0),
                                        stop=(ht == HT - 1),
                                    )
                    for ftl in range(FPG):
                        ft = fgi * FPG + ftl
                        for th in range(TH):
                            tsl = slice(th * NT, (th + 1) * NT)
                            tmp = tmpp.tile([P, NT], _F32, tag="tmp")
                            if ACT_MODE == "silu":
                                nc.scalar.activation(
                                    tmp[:], gate_ps[ftl, th][:],
                                    mybir.ActivationFunctionType.Silu,
                                )
                            else:
                                nc.scalar.activation(
                                    tmp[:], gate_ps[ftl, th][:],
                                    mybir.ActivationFunctionType.Sigmoid,
                                )
                                nc.vector.tensor_mul(
                                    tmp[:], tmp[:], gate_ps[ftl, th][:]
                                )
                            nc.vector.tensor_mul(
                                hidden[ft][:, tsl], tmp[:], up_ps[ftl, th][:]
                            )

                # Down projection: outT[h,t] accumulating over all 16 f-tiles.
                wd_v = wd_d[el].rearrange("(a p) h -> p a h", p=P)
                for hg in range(HGS):
                    hsl = slice(hg * P, (hg + 1) * P)
                    wdt = wdp.tile([P, FT, P], _F32R, tag="wd")
                    nc.sync.dma_start(wdt[:], wd_v[:, :, hsl])
                    ops = [psp.tile([P, NT], _F32, tag="ps", name="dn_ps") for _ in range(TH)]
                    for ft in range(FT):
                        lhsT = wdt[:, ft, :]
                        for th in range(TH):
                            nc.tensor.matmul(
                                ops[th][:],
                                lhsT,
                                hidden[ft][:, th * NT:(th + 1) * NT],
                                start=(ft == 0),
                                stop=(ft == FT - 1),
                            )
                    osb = osbp.tile([P, TPE], _F32, tag="osb")
                    for th in range(TH):
                        nc.vector.tensor_copy(osb[:, th * NT:(th + 1) * NT], ops[th][:])
                    # Stores go out on the ACT HWDGE ring so they never queue
                    # behind pending weight loads on the SP ring.
                    nc.scalar.dma_start(out_d[el, hsl, :], osb[:])
    return nc


def get_nc():
    if "nc" not in _CACHE:
        nc = _build_nc()
        nc.finalize()
        _CACHE["nc"] = nc
    return _CACHE["nc"]


def make_in_maps(x, w_gate, w_up, w_down):
    in_maps = []
    for c in range(NCORES):
        e0 = c * EPC
        xs = x[e0 * TPE:(e0 + EPC) * TPE].reshape(EPC, TPE, H)
        in_maps.append({
            "xt": np.ascontiguousarray(xs.transpose(0, 2, 1)),
            "wg": np.ascontiguousarray(w_gate[e0:e0 + EPC].transpose(0, 2, 1)),
            "wu": np.ascontiguousarray(w_up[e0:e0 + EPC].transpose(0, 2, 1)),
            "wd": np.ascontiguousarray(w_down[e0:e0 + EPC].transpose(0, 2, 1)),
        })
    return in_maps


def _numpy_fallback(x, w_gate, w_up, w_down, counts):
    out = np.empty((x.shape[0], w_down.shape[1]), np.float32)
    o = 0
    for e in range(len(counts)):
        n = int(counts[e])
        xi = x[o:o + n]
        gate = xi @ w_gate[e].T
        up = xi @ w_up[e].T
        hidden = (gate / (1.0 + np.exp(-gate))) * up
        out[o:o + n] = hidden @ w_down[e].T
        o += n
    return out


def kernel(x, w_gate, w_up, w_down, tokens_per_expert):
    global LAST_RESULTS
    x = np.asarray(x, dtype=np.float32)
    w_gate = np.asarray(w_gate, dtype=np.float32)
    w_up = np.asarray(w_up, dtype=np.float32)
    w_down = np.asarray(w_down, dtype=np.float32)
    counts = np.asarray(tokens_per_expert).astype(np.int64)

    if not (counts.shape == (E,) and np.all(counts == TPE)):
        # Non-uniform routing: the compiled program is shaped for the
        # uniform split the reference generator produces.
        return _numpy_fallback(x, w_gate, w_up, w_down, counts)

    nc = get_nc()
    res = run_bass_kernel_spmd(
        nc, make_in_maps(x, w_gate, w_up, w_down), list(range(NCORES)),
        trace=TRACE, **TRACE_KW,
    )
    LAST_RESULTS = res
    out = np.empty((T, H), np.float32)
    for c in range(NCORES):
        o = res.results[c]["outT"]  # [EPC, H, TPE]
        for el in range(EPC):
            t0 = (c * EPC + el) * TPE
            out[t0:t0 + TPE] = o[el].T
    return out
